# revision 1
# baseline (speedup 1.0000x reference)
"""Trainium2 Bass kernel for nn_Compositional: sigmoid(sum(er*ea*eb, -1)).

  ea = x @ W_ent.T   [N, D]
  eb = y @ W_ent.T   [N, D]
  er = r @ W_rel.T   [N, D]
  out = sigmoid(sum_d er*ea*eb)  [N, 1]

Sharding: data-parallel over N across 8 cores (512 rows each), weights
replicated.

Host-side prep (not on the device critical path): cast everything to bf16
(rel err ~6e-3, well inside the 2e-2 gate) and pre-transpose so every
device-side operand already has the contraction dim (e / rel-dim) on
partitions:
  xT [E, 512], yT [E, 512], rT [R, 512], W_entT [E, D], W_relT [R, D].

Per-core device plan (everything computed transposed: [D, n] with D on
partitions; no PE transposes, no PSUM->SBUF copies):
  - er first: 8 accumulating matmuls from W_relT/rT (also ramps up PE).
  - Main loop over 128 e-chunks in groups: per chunk, 4 accumulating
    matmuls (ea/eb x 2 d-halves), lhsT = W_entT chunk [128e, 128d],
    rhs = xT/yT chunk [128e, 512n], PSUM f32 accumulation. W/x/y all
    stream through ring tiles (W is consumed group-locally, so no
    resident W tile -> no WAR stalls on the DMA stream).
  - Tail groups are small with dedicated buffers so the last transfers
    are never blocked, then prod = ea*er*eb on DVE, partition-reduce via
    ones-matmul, sigmoid on ACT, DMA out.
bf16 halves DMA bytes vs f32 (~43MB/core) and runs the PE at full rate.
"""
import os

import numpy as np

# Full-problem constants (hardcoded; kernel.py must be self-contained).
N, E, R, D = 4096, 16384, 512, 256
NCORES = 8
NC_N = N // NCORES      # 512 rows per core
NCHUNK = E // 128       # 128 contraction chunks
DH = D // 128           # 2 d-halves
RC = R // 128           # 4 rel chunks

# Streaming granularity: W flows in 8-chunk groups; x/y flow in 2-chunk
# pieces (ea matmuls before eb per pair). In the FINAL group all x pieces
# stream before all y pieces: ea then closes during the y tail, letting the
# epilogue's t = ea*er products precompute on DVE, and the last transfers
# gate only a couple of eb matmuls. Tail pieces get dedicated buffers so
# ring WAR can never stall the end of the stream.
WG = 8                  # chunks per W DMA group
PIECE = 2               # chunks per x/y DMA piece
NWG = NCHUNK // WG      # 16 W groups
NPAIR = WG // PIECE     # x/y piece-pairs per W group
TAIL_SPLIT = 4                 # chunks of the final group in x-then-y order
TAIL_PIECES = [2, 1, 1]        # piece sizes within that x-then-y span
assert sum(TAIL_PIECES) == TAIL_SPLIT

_CACHE = {}


def _build():
    import concourse.mybir as mybir
    import concourse.tile as tile
    from concourse import bacc

    F32 = mybir.dt.float32
    BF16 = mybir.dt.bfloat16
    MUL = mybir.AluOpType.mult

    nc = bacc.Bacc("TRN2", target_bir_lowering=False)

    xT_d = nc.dram_tensor("xT", [E, NC_N], BF16, kind="ExternalInput")
    yT_d = nc.dram_tensor("yT", [E, NC_N], BF16, kind="ExternalInput")
    rT_d = nc.dram_tensor("rT", [R, NC_N], BF16, kind="ExternalInput")
    wentT_d = nc.dram_tensor("wentT", [E, D], BF16, kind="ExternalInput")
    wrelT_d = nc.dram_tensor("wrelT", [R, D], BF16, kind="ExternalInput")
    out_d = nc.dram_tensor("out", [NC_N, 1], F32, kind="ExternalOutput")

    with tile.TileContext(nc) as tc:
        with (
            tc.tile_pool(name="const", bufs=1) as cpool,
            tc.tile_pool(name="stream", bufs=1) as pool,
            tc.tile_pool(name="psum", bufs=1, space="PSUM") as psum,
        ):
            # ---- constants ----
            ones_f = cpool.tile([128, 1], F32)
            nc.gpsimd.memset(ones_f[:], 1.0)
            ones_b = cpool.tile([128, 1], BF16)
            nc.vector.tensor_copy(ones_b[:], ones_f[:])

            # ---- resident tensors (written once) ----
            wrelT_sb = cpool.tile([128, RC, D], BF16)      # [p_in, pchunk, d]
            rT_sb = cpool.tile([128, RC, NC_N], BF16)      # [p_in, pchunk, n]
            ert_sb = cpool.tile([128, DH, NC_N], F32)      # [d_in, dh, n]

            # ---- PSUM accumulators (persist through main loop) ----
            ea_ps = [
                psum.tile([128, NC_N], F32, tag=f"ea{dh}", bufs=1, name=f"ea{dh}")
                for dh in range(DH)
            ]
            eb_ps = [
                psum.tile([128, NC_N], F32, tag=f"eb{dh}", bufs=1, name=f"eb{dh}")
                for dh in range(DH)
            ]
            er_ps = [
                psum.tile([128, NC_N], F32, tag=f"er{dh}", bufs=1, name=f"er{dh}")
                for dh in range(DH)
            ]

            # ---- rel phase: tiny loads + er matmuls (warms up PE) ----
            nc.sync.dma_start(
                wrelT_sb[:], wrelT_d.rearrange("(c p) d -> p c d", p=128)
            )
            nc.sync.dma_start(
                rT_sb[:], rT_d.rearrange("(c p) n -> p c n", p=128)
            )
            for pc in range(RC):
                for dh in range(DH):
                    nc.tensor.matmul(
                        er_ps[dh][:],
                        wrelT_sb[:, pc, dh * 128 : (dh + 1) * 128],
                        rT_sb[:, pc, :],
                        start=(pc == 0),
                        stop=(pc == RC - 1),
                    )
            # stage er in SBUF: DVE tensor_tensor can read only one PSUM
            # input, and this also runs early, off the critical path (ACT).
            for dh in range(DH):
                nc.scalar.copy(ert_sb[:, dh, :], er_ps[dh][:])

            # ---- main loop ----
            def mm(acc, wg, src, k, c_w, c_s, dh):
                nc.tensor.matmul(
                    acc[:],
                    wg[:, c_w, dh * 128 : (dh + 1) * 128],
                    src[:, c_s, :],
                    start=(k == 0),
                    stop=(k == NCHUNK - 1),
                )

            def load(tag, bufs, dram, k0, pc):
                t = pool.tile(
                    [128, pc, NC_N], BF16, tag=tag, bufs=bufs, name=tag
                )
                nc.sync.dma_start(
                    t[:],
                    dram[k0 * 128 : (k0 + pc) * 128, :].rearrange(
                        "(c p) n -> p c n", p=128
                    ),
                )
                return t

            def load_w(g):
                wg = pool.tile([128, WG, D], BF16, tag="wg", bufs=4, name="wg")
                nc.sync.dma_start(
                    wg[:],
                    wentT_d[g * WG * 128 : (g + 1) * WG * 128, :].rearrange(
                        "(c p) d -> p c d", p=128
                    ),
                )
                return wg

            for g in range(NWG):
                if g < NWG - 1:
                    wg = load_w(g)
                else:
                    # final group: W streams in tapered pieces placed just
                    # ahead of their consumers, so no big W transfer gates a
                    # 32-matmul burst at the very end
                    wg = pool.tile(
                        [128, WG, D], BF16, tag="wg_tail", bufs=1, name="wg"
                    )
                    nc.sync.dma_start(
                        wg[:, : WG - TAIL_SPLIT, :],
                        wentT_d[
                            g * WG * 128 : (g * WG + WG - TAIL_SPLIT) * 128, :
                        ].rearrange("(c p) d -> p c d", p=128),
                    )
                npair = NPAIR if g < NWG - 1 else (WG - TAIL_SPLIT) // PIECE
                # x/y pair-interleaved pieces; ea matmuls lead eb per pair
                for pp in range(npair):
                    c0 = pp * PIECE
                    k0 = g * WG + c0
                    xp = load("xp", 8, xT_d, k0, PIECE)
                    yp = load("yp", 8, yT_d, k0, PIECE)
                    for c in range(PIECE):
                        for dh in range(DH):
                            mm(ea_ps[dh], wg, xp, k0 + c, c0 + c, c, dh)
                    for c in range(PIECE):
                        for dh in range(DH):
                            mm(eb_ps[dh], wg, yp, k0 + c, c0 + c, c, dh)
                if g == NWG - 1:
                    # tail W pieces, emitted before the x tail
                    tw0 = WG - TAIL_SPLIT
                    for c0, pc in [(tw0, 2), (tw0 + 2, 1), (tw0 + 3, 1)]:
                        nc.sync.dma_start(
                            wg[:, c0 : c0 + pc, :],
                            wentT_d[
                                (g * WG + c0) * 128 : (g * WG + c0 + pc) * 128, :
                            ].rearrange("(c p) d -> p c d", p=128),
                        )
                    # final span: all x pieces stream (and their ea matmuls
                    # run) before any y; ea closes during the y tail
                    pieces, c0 = [], WG - TAIL_SPLIT
                    for pc in TAIL_PIECES:
                        pieces.append((c0, pc))
                        c0 += pc
                    for c0, pc in pieces:
                        k0 = g * WG + c0
                        xp = load(f"xtail{c0}", 1, xT_d, k0, pc)
                        for c in range(pc):
                            for dh in range(DH):
                                mm(ea_ps[dh], wg, xp, k0 + c, c0 + c, c, dh)
                    # ea is closed: precompute t = ea*er on DVE while the y
                    # tail streams in
                    t_sbs = []
                    for dh in range(DH):
                        t_sb = pool.tile(
                            [128, NC_N], F32, tag=f"t_sb{dh}", bufs=1, name="t_sb"
                        )
                        nc.vector.tensor_tensor(
                            t_sb[:], ea_ps[dh][:], ert_sb[:, dh, :], MUL
                        )
                        t_sbs.append(t_sb)
                    for c0, pc in pieces:
                        k0 = g * WG + c0
                        yp = load(f"ytail{c0}", 1, yT_d, k0, pc)
                        for c in range(pc):
                            for dh in range(DH):
                                mm(eb_ps[dh], wg, yp, k0 + c, c0 + c, c, dh)

            # ---- epilogue: p = t*eb + partition-reduce + sigmoid ----
            # (t = ea*er precomputed above during the y tail; dh0 closes
            # first so its chain leads.)
            score_ps = psum.tile([1, NC_N], F32, tag="score", bufs=1, name="score")
            for dh in range(DH):
                p_sb = pool.tile(
                    [128, NC_N], BF16, tag=f"p_sb{dh}", bufs=1, name="p_sb"
                )
                nc.vector.tensor_tensor(p_sb[:], eb_ps[dh][:], t_sbs[dh][:], MUL)
                nc.tensor.matmul(
                    score_ps[:],
                    ones_b[:],
                    p_sb[:],
                    start=(dh == 0),
                    stop=(dh == DH - 1),
                )
            sig_sb = pool.tile([1, NC_N], F32, name="sig_sb")
            nc.scalar.activation(
                sig_sb[:], score_ps[:], mybir.ActivationFunctionType.Sigmoid
            )
            nc.sync.dma_start(out_d.rearrange("n o -> o n"), sig_sb[:])

    nc.compile()
    return nc


def _get_nc():
    if "nc" not in _CACHE:
        _CACHE["nc"] = _build()
    return _CACHE["nc"]


def kernel(x, y, r, W_ent, W_rel):
    import ml_dtypes
    from concourse.bass_utils import run_bass_kernel_spmd

    bf16 = ml_dtypes.bfloat16
    x_b = np.asarray(x, dtype=np.float32).astype(bf16)
    y_b = np.asarray(y, dtype=np.float32).astype(bf16)
    r_b = np.asarray(r, dtype=np.float32).astype(bf16)
    wentT = np.ascontiguousarray(np.asarray(W_ent, dtype=np.float32).astype(bf16).T)
    wrelT = np.ascontiguousarray(np.asarray(W_rel, dtype=np.float32).astype(bf16).T)

    nc = _get_nc()
    in_maps = [
        {
            "xT": np.ascontiguousarray(x_b[c * NC_N : (c + 1) * NC_N].T),
            "yT": np.ascontiguousarray(y_b[c * NC_N : (c + 1) * NC_N].T),
            "rT": np.ascontiguousarray(r_b[c * NC_N : (c + 1) * NC_N].T),
            "wentT": wentT,
            "wrelT": wrelT,
        }
        for c in range(NCORES)
    ]
    trace = bool(int(os.environ.get("KERNEL_TRACE", "0")))
    res = run_bass_kernel_spmd(
        nc, in_maps, core_ids=list(range(NCORES)), trace=trace
    )
    _CACHE["last_result"] = res
    out = np.concatenate([res.results[c]["out"] for c in range(NCORES)], axis=0)
    return out

